# revision 22
# baseline (speedup 1.0000x reference)
# Causal self-attention kernel for 8 Trainium2 NeuronCores (Bass/Tile).
#
# Sharding: core c -> batch b = c//4, head group g = c%4 (heads 4g..4g+3).
# Each core computes the qkv projection for its batch restricted to its heads
# (column-sharded Wqkv), rope, causal flash attention for its 4 heads, and a
# row-sharded output projection producing a partial [S, D] f32 output.  The
# host sums the 4 partials per batch and adds bout.
#
# Device-side notes:
#  * All matmul inputs are bf16 (fp32 matmul is 4x slower on the PE); all
#    accumulation is f32 in PSUM.
#  * x is pre-transposed on host to xT [D, S] so the contraction dim lands on
#    SBUF partitions without any on-device transpose.
#  * q/k are produced directly transposed (qT/kT [dims, S]).  Within each head
#    the dims are permuted to [evens(32), odds(32)] so rope becomes
#    rot = x*P + swap32(x)*Q with host-built per-row tables P/Q; swap32 is two
#    32-partition-block exchanges done with SBUF->SBUF DMAs.
#  * Scores are computed transposed, sT[k, q], with the k-side stationary
#    operand zero-padded to K=128: K=64 matmuls never warm the PE clock gate
#    (HAM) and run at half clock forever.
#  * Causal masking of diagonal score tiles is one extra accumulating matmul
#    (-30000 * upper-triangle) -- no cross-engine dependency.
#  * Softmax without max-subtraction (scores ~ N(0,1); exp is safe in f32):
#    p = exp(s/8) straight out of PSUM on the scalar engine, bf16 out.
#  * v_ext [k, 65] carries a ones-column so the PV matmul accumulates the
#    softmax denominator as row 64 of oT [65, q].  oT is evacuated to SBUF
#    immediately (frees PSUM fast); reciprocal + gpsimd partition_broadcast +
#    multiply run off the PE's critical path.
#  * Attention runs in q-halves so oT is [65, 1024] = 2 PSUM banks and can
#    double-buffer; the PV matmul of iteration n is emitted after the scores
#    matmul of iteration n+1 so the PE never waits on the exp.
#  * Output projection: y[q, n] = sum_d aT[d, q] * Wout[d, n], q-tiled.

import numpy as np
import ml_dtypes

import concourse.bass as bass
import concourse.tile as tile
import concourse.mybir as mybir
from concourse import bacc
from concourse.bass import ts, ds
from concourse.bass_utils import run_bass_kernel_spmd

BF16 = mybir.dt.bfloat16
F32 = mybir.dt.float32
AF = mybir.ActivationFunctionType
ALU = mybir.AluOpType

B, S, D = 2, 2048, 1024
H, HD = 16, 64
NCORES = 8
HPC = 4            # heads per core
NT = S // 128      # 16 seq tiles
SCALE = HD ** -0.5
MASK_NEG = -30000.0

# Module-level knobs / results (used by test.py).
TRACE = False
LAST_RESULTS = None


def _body(ctx, tc, ins, outs):
    nc = tc.nc
    xT, wqk, wv, wout, bqk, bvb, ropeP, ropeQ = ins
    (y,) = outs

    # ---- SBUF pools ----
    p_x = ctx.enter_context(tc.tile_pool(name="x", bufs=1))
    p_w = ctx.enter_context(tc.tile_pool(name="w", bufs=1))
    p_cst = ctx.enter_context(tc.tile_pool(name="cst", bufs=1))
    p_qk = ctx.enter_context(tc.tile_pool(name="qk", bufs=1))
    p_vx = ctx.enter_context(tc.tile_pool(name="vx", bufs=1))
    p_aT = ctx.enter_context(tc.tile_pool(name="aT", bufs=1))
    p_tmp = ctx.enter_context(tc.tile_pool(name="tmp", bufs=3))
    p_p = ctx.enter_context(tc.tile_pool(name="p", bufs=6))
    p_r = ctx.enter_context(tc.tile_pool(name="r", bufs=2))
    p_y = ctx.enter_context(tc.tile_pool(name="y", bufs=4))

    # ---- load inputs (x and weight DMAs interleaved) ----
    x_sb, wqk_sb, wv_sb = [], [], []
    for kc in range(8):
        t = p_x.tile([128, S], BF16, tag=f"x{kc}", name=f"x{kc}")
        nc.sync.dma_start(t[:, :], xT[ts(kc, 128), :])
        x_sb.append(t)
        t = p_w.tile([128, 512], BF16, tag=f"wqk{kc}", name=f"wqk{kc}")
        nc.sync.dma_start(t[:, :], wqk[ts(kc, 128), :])
        wqk_sb.append(t)
        t = p_w.tile([128, 256], BF16, tag=f"wv{kc}", name=f"wv{kc}")
        nc.sync.dma_start(t[:, :], wv[ts(kc, 128), :])
        wv_sb.append(t)
    wout_sb = []
    for kc in range(2):
        t = p_w.tile([128, D], BF16, tag=f"wout{kc}", name=f"wout{kc}")
        nc.sync.dma_start(t[:, :], wout[ts(kc, 128), :])
        wout_sb.append(t)
    tabP = p_cst.tile([128, S], BF16, tag="tabP")
    nc.sync.dma_start(tabP[:, :], ropeP[:, :])
    tabQ = p_cst.tile([128, S], BF16, tag="tabQ")
    nc.sync.dma_start(tabQ[:, :], ropeQ[:, :])
    bqk_sb = []
    for mc in range(4):
        t = p_cst.tile([128, 1], F32, tag=f"bqk{mc}", name=f"bqk{mc}")
        nc.sync.dma_start(t[:, :], bqk[ts(mc, 128)].rearrange("(p o) -> p o", o=1))
        bqk_sb.append(t)
    bvb_sb = p_cst.tile([128, 256], F32, tag="bvb")
    nc.sync.dma_start(bvb_sb[:, :], bvb[:, :])

    # constants for the matmul-based causal mask of diagonal score tiles:
    # s_diag += (neg_ident.T @ upper01) = -30000 where k > q.
    # (only is_ge is implemented for affine_select in this compiler)
    ones_t = p_cst.tile([128, 128], BF16, tag="ones")
    nc.vector.memset(ones_t[:, :], 1.0)
    upper01 = p_cst.tile([128, 128], BF16, tag="upper01")
    nc.gpsimd.affine_select(upper01[:, :], ones_t[:, :], pattern=[[-1, 128]],
                            compare_op=ALU.is_ge, fill=0.0, base=-1,
                            channel_multiplier=1)   # keep where k - q - 1 >= 0
    lower_t = p_cst.tile([128, 128], BF16, tag="lower")
    nc.gpsimd.affine_select(lower_t[:, :], ones_t[:, :], pattern=[[1, 128]],
                            compare_op=ALU.is_ge, fill=0.0, base=0,
                            channel_multiplier=-1)  # keep where q - k >= 0
    ident_t = p_cst.tile([128, 128], BF16, tag="ident")
    nc.gpsimd.affine_select(ident_t[:, :], lower_t[:, :], pattern=[[-1, 128]],
                            compare_op=ALU.is_ge, fill=0.0, base=0,
                            channel_multiplier=1)   # and k - q >= 0
    neg_ident = p_cst.tile([128, 128], BF16, tag="neg_ident")
    nc.vector.tensor_scalar_mul(neg_ident[:, :], ident_t[:, :], MASK_NEG)

    qk_sb = []   # [q01, q23, k01, k23], bf16 [128, S] each (post-rope)
    for mc in range(4):
        qk_sb.append(p_qk.tile([128, S], BF16, tag=f"qkT{mc}", name=f"qkT{mc}"))
    # per-head zero-padded kT [128, S]: only head h's 64 rows (at offset
    # 64*(h%2)) are nonzero, for full-K scores matmuls (K=64 never warms HAM).
    kpad_sb = []
    for h in range(HPC):
        t = p_qk.tile([128, S], BF16, tag=f"kpad{h}", name=f"kpad{h}")
        nc.vector.memset(t[64 * (1 - h % 2):64 * (1 - h % 2) + 64, :], 0.0)
        kpad_sb.append(t)
    vx_sb = [None] * NT  # [128, 4*65] bf16: per head 64 v-cols + ones col
    aT_sb = [p_aT.tile([128, S], BF16, tag=f"aT{i}", name=f"aT{i}")
             for i in range(2)]

    # ---- phase A: projections + rope ----
    with tc.tile_pool(name="ps_qk", bufs=4, space="PSUM") as ps_qk, \
         tc.tile_pool(name="ps_v", bufs=2, space="PSUM") as ps_v:
        with nc.named_scope("warmup"):
            # ~4us of dense K=128 matmuls (discarded) flips the PE clock
            # gate to 2.4 GHz while the x DMAs are still in flight
            wu = ps_v.tile([128, 512], F32, tag="v", name="warmup")
            for r in range(18):
                nc.tensor.matmul(wu[:, :], wqk_sb[0][:, 0:128],
                                 wqk_sb[0][:, :], start=(r == 0),
                                 stop=(r == 17), skip_group_check=True)
        def v_group(st_range):
            for st in st_range:
                v_ps = ps_v.tile([128, 256], F32, tag="v", name=f"v{st}")
                for kc in range(8):
                    nc.tensor.matmul(
                        v_ps[:, :],
                        x_sb[kc][:, ts(st, 128)],
                        wv_sb[kc][:, :],
                        start=(kc == 0), stop=(kc == 7))
                vx_t = p_vx.tile([128, HPC * 65], BF16, tag=f"vx{st}",
                                 name=f"vx{st}")
                vv = vx_t.rearrange("p (h c) -> p h c", c=65)
                nc.vector.memset(vv[:, :, 64:65], 1.0)
                nc.vector.tensor_add(
                    vv[:, :, 0:64],
                    v_ps.rearrange("p (h c) -> p h c", c=64)[:, :, :],
                    bvb_sb.rearrange("p (h c) -> p h c", c=64)[:, :, :])
                vx_sb[st] = vx_t

        with nc.named_scope("qk_proj"):
            for mc in (0, 2, -1, 1, 3, -2):  # -1/-2 = v-projection groups
                if mc == -1:
                    with nc.named_scope("v_proj_a"):
                        v_group(range(0, 8))
                    continue
                if mc == -2:
                    with nc.named_scope("v_proj_b"):
                        v_group(range(8, 16))
                    continue
                qk_pss = [ps_qk.tile([128, 512], F32, tag="qk",
                                     name=f"qk{mc}_{i}") for i in range(4)]
                if mc == 0:
                    # ns-outer: the first matmuls only need the first x
                    # chunks (input DMAs still in flight)
                    for ns in range(4):
                        for kc in range(8):
                            nc.tensor.matmul(
                                qk_pss[ns][:, :],
                                wqk_sb[kc][:, ts(mc, 128)],
                                x_sb[kc][:, ts(ns, 512)],
                                start=(kc == 0), stop=(kc == 7))
                else:
                    for kc in range(8):   # stationary wqk(kc, mc) reused
                        for ns in range(4):
                            nc.tensor.matmul(
                                qk_pss[ns][:, :],
                                wqk_sb[kc][:, ts(mc, 128)],
                                x_sb[kc][:, ts(ns, 512)],
                                start=(kc == 0), stop=(kc == 7))
                for ns in range(4):
                    qk_ps = qk_pss[ns]
                    raw = p_tmp.tile([128, 512], BF16, tag="raw")
                    nc.vector.tensor_scalar_add(raw[:, :], qk_ps[:, :],
                                                bqk_sb[mc][:, :])
                    # swap32: exchange adjacent 32-partition blocks
                    swp = p_tmp.tile([128, 512], BF16, tag="swp")
                    for blk, sb in ((0, 32), (32, 0), (64, 96), (96, 64)):
                        nc.sync.dma_start(swp[blk:blk + 32, :],
                                          raw[sb:sb + 32, :])
                    t1 = p_tmp.tile([128, 512], BF16, tag="t1")
                    nc.vector.tensor_mul(t1[:, :], swp[:, :], tabQ[:, ts(ns, 512)])
                    t2 = p_tmp.tile([128, 512], BF16, tag="t2")
                    nc.vector.tensor_mul(t2[:, :], raw[:, :], tabP[:, ts(ns, 512)])
                    if mc < 2:
                        nc.vector.tensor_add(
                            qk_sb[mc][:, ts(ns, 512)], t1[:, :], t2[:, :])
                    else:
                        hp = mc - 2
                        for hr in range(2):
                            nc.vector.tensor_add(
                                kpad_sb[2 * hp + hr][64 * hr:64 * hr + 64,
                                                     ts(ns, 512)],
                                t1[64 * hr:64 * hr + 64, :],
                                t2[64 * hr:64 * hr + 64, :])
    # ---- phase B: attention ----
    with tc.tile_pool(name="ps_s", bufs=2, space="PSUM") as ps_s, \
         tc.tile_pool(name="ps_o", bufs=2, space="PSUM") as ps_o:
        pend = [None]   # (p_t, j, q0, w, oT, qlo, h)

        def emit_pv(pv):
            p_t, j, q0, w, oT, qlo, h = pv
            c0 = (q0 - qlo) * 128
            pos = c0
            while pos < c0 + w:
                nxt = min((pos // 512 + 1) * 512, c0 + w)
                gbank = (qlo * 128 + pos) // 512
                nc.tensor.matmul(
                    oT[:, ds(pos, nxt - pos)],
                    vx_sb[j][:, ds(65 * h, 65)],
                    p_t[:, ds(pos - c0, nxt - pos)],
                    start=(j == 0), stop=(j == 4 * gbank + 3),
                    skip_group_check=True)
                pos = nxt

        def finish_half(h, hp, hr, half, oT):
            # Evacuate oT to SBUF right away so the PSUM banks free without
            # waiting on the (slow, single-lane) reciprocal; then normalize
            # from the copy in chunks, fully off the PE's path.
            num = p_r.tile([64, 1024], BF16, tag="num", name=f"num{h}_{half}")
            nc.vector.tensor_copy(num[:, :], oT[0:64, :])
            den = p_r.tile([1, 1024], F32, tag="den", name=f"den{h}_{half}")
            nc.vector.tensor_copy(den[:, :], oT[64:65, :])
            csz = 256 if (h == HPC - 1 and half == 1) else 512
            for c in range(1024 // csz):
                r_t = p_r.tile([1, 512], F32, tag="r", name=f"r{h}_{half}_{c}")
                nc.vector.reciprocal(r_t[:, 0:csz], den[:, ds(csz * c, csz)])
                rb_t = p_r.tile([64, 512], F32, tag="rb",
                                name=f"rb{h}_{half}_{c}")
                nc.gpsimd.partition_broadcast(rb_t[:, 0:csz], r_t[:, 0:csz])
                nc.vector.tensor_mul(
                    aT_sb[hp][64 * hr:64 * hr + 64,
                              ds(1024 * half + csz * c, csz)],
                    num[:, ds(csz * c, csz)], rb_t[:, 0:csz])

        finishes = []
        for h in range(HPC):
            hp, hr = h // 2, h % 2
            qT = qk_sb[hp]
            kT = kpad_sb[h]
            with nc.named_scope(f"attn_h{h}"):
                for half in range(2):
                    qlo, qhi = 8 * half, 8 * half + 8   # q-tile range
                    oT = ps_o.tile([65, 1024], F32, tag="oT",
                                   name=f"oT{h}_{half}")
                    for j in range(qhi):
                        q0 = max(j, qlo)
                        w = (qhi - q0) * 128
                        s_ps = ps_s.tile([128, 1024], F32, tag="s")
                        diag = (q0 == j)
                        for n0 in range(0, w, 512):
                            nn = min(512, w - n0)
                            has_mask = diag and n0 == 0
                            nc.tensor.matmul(
                                s_ps[:, ds(n0, nn)],
                                kT[:, ts(j, 128)],
                                qT[:, ds(q0 * 128 + n0, nn)],
                                start=True, stop=not has_mask,
                                skip_group_check=True)
                            if has_mask:
                                nc.tensor.matmul(
                                    s_ps[:, 0:128], neg_ident[:, :],
                                    upper01[:, :], start=False, stop=True,
                                    skip_group_check=True)
                        p_t = p_p.tile([128, 1024], BF16, tag="p")
                        nc.scalar.activation(
                            p_t[:, 0:w], s_ps[:, 0:w], AF.Exp, scale=SCALE)
                        if pend[0] is not None:
                            emit_pv(pend[0])
                            for fin in finishes:
                                fin()
                            finishes = []
                        pend[0] = (p_t, j, q0, w, oT, qlo, h)
                    finishes.append(
                        lambda h=h, hp=hp, hr=hr, half=half, oT=oT:
                            finish_half(h, hp, hr, half, oT))
        emit_pv(pend[0])
        for fin in finishes:
            fin()

    # ---- phase C: output projection ----
    with tc.tile_pool(name="ps_y", bufs=4, space="PSUM") as ps_y:
        with nc.named_scope("y_proj"):
            for qt in range(NT):
                for nh in range(2):
                    y_ps = ps_y.tile([128, 512], F32, tag="y")
                    for kc in range(2):
                        nc.tensor.matmul(
                            y_ps[:, :],
                            aT_sb[kc][:, ts(qt, 128)],
                            wout_sb[kc][:, ts(nh, 512)],
                            start=(kc == 0), stop=(kc == 1))
                    y_sb = p_y.tile([128, 512], F32, tag="ysb")
                    if nh == 0:
                        nc.vector.tensor_copy(y_sb[:, :], y_ps[:, :])
                    else:
                        nc.scalar.copy(y_sb[:, :], y_ps[:, :])
                    nc.sync.dma_start(y[ts(qt, 128), ts(nh, 512)], y_sb[:, :])


def build():
    nc = bacc.Bacc("TRN2", target_bir_lowering=False, debug=False,
                   num_devices=NCORES)
    xT = nc.dram_tensor("xT", [D, S], BF16, kind="ExternalInput").ap()
    wqk = nc.dram_tensor("wqk", [D, 512], BF16, kind="ExternalInput").ap()
    wv = nc.dram_tensor("wv", [D, 256], BF16, kind="ExternalInput").ap()
    wout = nc.dram_tensor("wout", [256, D], BF16, kind="ExternalInput").ap()
    bqk = nc.dram_tensor("bqk", [512], F32, kind="ExternalInput").ap()
    bvb = nc.dram_tensor("bvb", [128, 256], F32, kind="ExternalInput").ap()
    ropeP = nc.dram_tensor("ropeP", [128, S], BF16, kind="ExternalInput").ap()
    ropeQ = nc.dram_tensor("ropeQ", [128, S], BF16, kind="ExternalInput").ap()
    y = nc.dram_tensor("y", [S, D], F32, kind="ExternalOutput").ap()

    from contextlib import ExitStack
    with tile.TileContext(nc) as tc:
        with ExitStack() as ctx:
            _body(ctx, tc, (xT, wqk, wv, wout, bqk, bvb, ropeP, ropeQ), (y,))
    nc.compile()
    return nc


_EVEN_ODD = np.concatenate([np.arange(0, HD, 2), np.arange(1, HD, 2)])


def make_core_inputs(x, rope_cos, rope_sin, Wqkv, bqkv, Wout, bout, core):
    """Build the per-core device input map (numpy, host-side sharding)."""
    b, g = core // HPC, core % HPC
    heads = [HPC * g + i for i in range(HPC)]
    bf = ml_dtypes.bfloat16

    xT = np.ascontiguousarray(x[b].T).astype(bf)

    # wqk columns: [q01, q23, k01, k23]; within each head [evens, odds]
    qcols, kcols = [], []
    for h in heads:
        qcols.append(Wqkv[:, 0 * D + 64 * h + _EVEN_ODD])
        kcols.append(Wqkv[:, 1 * D + 64 * h + _EVEN_ODD])
    wqk_np = np.concatenate(
        [qcols[0], qcols[1], qcols[2], qcols[3],
         kcols[0], kcols[1], kcols[2], kcols[3]], axis=1)
    bq = [bqkv[0 * D + 64 * h + _EVEN_ODD] for h in heads]
    bk = [bqkv[1 * D + 64 * h + _EVEN_ODD] for h in heads]
    bqk_np = np.concatenate([bq[0], bq[1], bq[2], bq[3],
                             bk[0], bk[1], bk[2], bk[3]])

    wv_np = np.concatenate(
        [Wqkv[:, 2 * D + 64 * h:2 * D + 64 * h + 64] for h in heads], axis=1)
    bv = np.concatenate(
        [bqkv[2 * D + 64 * h:2 * D + 64 * h + 64] for h in heads])
    bvb_np = np.tile(bv[None, :], (128, 1)).astype(np.float32)

    wout_np = np.concatenate(
        [Wout[64 * h:64 * h + 64, :] for h in heads], axis=0)

    cosT = np.ascontiguousarray(rope_cos.T).astype(np.float32)  # [32, S]
    sinT = np.ascontiguousarray(rope_sin.T).astype(np.float32)
    ropeP_np = np.tile(np.concatenate([cosT, cosT], axis=0), (2, 1))
    ropeQ_np = np.tile(np.concatenate([-sinT, sinT], axis=0), (2, 1))

    return {
        "xT": xT,
        "wqk": np.ascontiguousarray(wqk_np).astype(bf),
        "wv": np.ascontiguousarray(wv_np).astype(bf),
        "wout": np.ascontiguousarray(wout_np).astype(bf),
        "bqk": bqk_np.astype(np.float32),
        "bvb": bvb_np,
        "ropeP": np.ascontiguousarray(ropeP_np).astype(bf),
        "ropeQ": np.ascontiguousarray(ropeQ_np).astype(bf),
    }


_NC_CACHE = None


def kernel(x, rope_cos, rope_sin, Wqkv, bqkv, Wout, bout):
    global _NC_CACHE, LAST_RESULTS
    x = np.asarray(x, dtype=np.float32)
    rope_cos = np.asarray(rope_cos, dtype=np.float32)
    rope_sin = np.asarray(rope_sin, dtype=np.float32)
    Wqkv = np.asarray(Wqkv, dtype=np.float32)
    bqkv = np.asarray(bqkv, dtype=np.float32)
    Wout = np.asarray(Wout, dtype=np.float32)
    bout = np.asarray(bout, dtype=np.float32)

    if _NC_CACHE is None:
        _NC_CACHE = build()
    nc = _NC_CACHE

    in_maps = [
        make_core_inputs(x, rope_cos, rope_sin, Wqkv, bqkv, Wout, bout, c)
        for c in range(NCORES)
    ]
    res = run_bass_kernel_spmd(nc, in_maps, core_ids=list(range(NCORES)),
                               trace=TRACE)
    LAST_RESULTS = res

    out = np.zeros((B, S, D), dtype=np.float32)
    for c in range(NCORES):
        out[c // HPC] += res.results[c]["y"]
    out += bout[None, None, :]
    return out


# revision 23
# speedup vs baseline: 1.0213x; 1.0213x over previous
# Causal self-attention kernel for 8 Trainium2 NeuronCores (Bass/Tile).
#
# Sharding: core c -> batch b = c//4, head group g = c%4 (heads 4g..4g+3).
# Each core computes the qkv projection for its batch restricted to its heads
# (column-sharded Wqkv), rope, causal flash attention for its 4 heads, and a
# row-sharded output projection producing a partial [S, D] f32 output.  The
# host sums the 4 partials per batch and adds bout.
#
# Device-side notes:
#  * All matmul inputs are bf16 (fp32 matmul is 4x slower on the PE); all
#    accumulation is f32 in PSUM.
#  * x is pre-transposed on host to xT [D, S] so the contraction dim lands on
#    SBUF partitions without any on-device transpose.
#  * q/k are produced directly transposed (qT/kT [dims, S]).  Within each head
#    the dims are permuted to [evens(32), odds(32)] so rope becomes
#    rot = x*P + swap32(x)*Q with host-built per-row tables P/Q; swap32 is two
#    32-partition-block exchanges done with SBUF->SBUF DMAs.
#  * Scores are computed transposed, sT[k, q], with the k-side stationary
#    operand zero-padded to K=128: K=64 matmuls never warm the PE clock gate
#    (HAM) and run at half clock forever.
#  * Causal masking of diagonal score tiles is one extra accumulating matmul
#    (-30000 * upper-triangle) -- no cross-engine dependency.
#  * Softmax without max-subtraction (scores ~ N(0,1); exp is safe in f32):
#    p = exp(s/8) straight out of PSUM on the scalar engine, bf16 out.
#  * v_ext [k, 65] carries a ones-column so the PV matmul accumulates the
#    softmax denominator as row 64 of oT [65, q].  oT is evacuated to SBUF
#    immediately (frees PSUM fast); reciprocal + gpsimd partition_broadcast +
#    multiply run off the PE's critical path.
#  * Attention runs in q-halves so oT is [65, 1024] = 2 PSUM banks and can
#    double-buffer; the PV matmul of iteration n is emitted after the scores
#    matmul of iteration n+1 so the PE never waits on the exp.
#  * Output projection: y[q, n] = sum_d aT[d, q] * Wout[d, n], q-tiled.

import numpy as np
import ml_dtypes

import concourse.bass as bass
import concourse.tile as tile
import concourse.mybir as mybir
from concourse import bacc
from concourse.bass import ts, ds
from concourse.bass_utils import run_bass_kernel_spmd

BF16 = mybir.dt.bfloat16
F32 = mybir.dt.float32
AF = mybir.ActivationFunctionType
ALU = mybir.AluOpType

B, S, D = 2, 2048, 1024
H, HD = 16, 64
NCORES = 8
HPC = 4            # heads per core
NT = S // 128      # 16 seq tiles
SCALE = HD ** -0.5
MASK_NEG = -30000.0

# Module-level knobs / results (used by test.py).
TRACE = False
LAST_RESULTS = None


def _body(ctx, tc, ins, outs):
    nc = tc.nc
    xT, wqk, wv, wout, bqk, bvb, ropeP, ropeQ = ins
    (y,) = outs

    # ---- SBUF pools ----
    p_x = ctx.enter_context(tc.tile_pool(name="x", bufs=1))
    p_w = ctx.enter_context(tc.tile_pool(name="w", bufs=1))
    p_cst = ctx.enter_context(tc.tile_pool(name="cst", bufs=1))
    p_qk = ctx.enter_context(tc.tile_pool(name="qk", bufs=1))
    p_vx = ctx.enter_context(tc.tile_pool(name="vx", bufs=1))
    p_aT = ctx.enter_context(tc.tile_pool(name="aT", bufs=1))
    p_tmp = ctx.enter_context(tc.tile_pool(name="tmp", bufs=3))
    p_p = ctx.enter_context(tc.tile_pool(name="p", bufs=6))
    p_r = ctx.enter_context(tc.tile_pool(name="r", bufs=2))
    p_y = ctx.enter_context(tc.tile_pool(name="y", bufs=4))

    # ---- load inputs (x and weight DMAs interleaved) ----
    x_sb, wqk_sb, wv_sb = [], [], []
    for kc in range(8):
        t = p_x.tile([128, S], BF16, tag=f"x{kc}", name=f"x{kc}")
        # issue x loads from the scalar engine's DMA queue: the sync
        # sequencer alone needs ~17us just to issue all input DMAs
        nc.scalar.dma_start(t[:, :], xT[ts(kc, 128), :])
        x_sb.append(t)
        t = p_w.tile([128, 512], BF16, tag=f"wqk{kc}", name=f"wqk{kc}")
        nc.sync.dma_start(t[:, :], wqk[ts(kc, 128), :])
        wqk_sb.append(t)
        t = p_w.tile([128, 256], BF16, tag=f"wv{kc}", name=f"wv{kc}")
        nc.sync.dma_start(t[:, :], wv[ts(kc, 128), :])
        wv_sb.append(t)
    wout_sb = []
    for kc in range(2):
        t = p_w.tile([128, D], BF16, tag=f"wout{kc}", name=f"wout{kc}")
        nc.sync.dma_start(t[:, :], wout[ts(kc, 128), :])
        wout_sb.append(t)
    tabP = p_cst.tile([128, S], BF16, tag="tabP")
    nc.sync.dma_start(tabP[:, :], ropeP[:, :])
    tabQ = p_cst.tile([128, S], BF16, tag="tabQ")
    nc.sync.dma_start(tabQ[:, :], ropeQ[:, :])
    bqk_sb = []
    for mc in range(4):
        t = p_cst.tile([128, 1], F32, tag=f"bqk{mc}", name=f"bqk{mc}")
        nc.sync.dma_start(t[:, :], bqk[ts(mc, 128)].rearrange("(p o) -> p o", o=1))
        bqk_sb.append(t)
    bvb_sb = p_cst.tile([128, 256], F32, tag="bvb")
    nc.sync.dma_start(bvb_sb[:, :], bvb[:, :])

    # constants for the matmul-based causal mask of diagonal score tiles:
    # s_diag += (neg_ident.T @ upper01) = -30000 where k > q.
    # (only is_ge is implemented for affine_select in this compiler)
    ones_t = p_cst.tile([128, 128], BF16, tag="ones")
    nc.vector.memset(ones_t[:, :], 1.0)
    upper01 = p_cst.tile([128, 128], BF16, tag="upper01")
    nc.gpsimd.affine_select(upper01[:, :], ones_t[:, :], pattern=[[-1, 128]],
                            compare_op=ALU.is_ge, fill=0.0, base=-1,
                            channel_multiplier=1)   # keep where k - q - 1 >= 0
    lower_t = p_cst.tile([128, 128], BF16, tag="lower")
    nc.gpsimd.affine_select(lower_t[:, :], ones_t[:, :], pattern=[[1, 128]],
                            compare_op=ALU.is_ge, fill=0.0, base=0,
                            channel_multiplier=-1)  # keep where q - k >= 0
    ident_t = p_cst.tile([128, 128], BF16, tag="ident")
    nc.gpsimd.affine_select(ident_t[:, :], lower_t[:, :], pattern=[[-1, 128]],
                            compare_op=ALU.is_ge, fill=0.0, base=0,
                            channel_multiplier=1)   # and k - q >= 0
    neg_ident = p_cst.tile([128, 128], BF16, tag="neg_ident")
    nc.vector.tensor_scalar_mul(neg_ident[:, :], ident_t[:, :], MASK_NEG)

    qk_sb = []   # [q01, q23, k01, k23], bf16 [128, S] each (post-rope)
    for mc in range(4):
        qk_sb.append(p_qk.tile([128, S], BF16, tag=f"qkT{mc}", name=f"qkT{mc}"))
    # per-head zero-padded kT [128, S]: only head h's 64 rows (at offset
    # 64*(h%2)) are nonzero, for full-K scores matmuls (K=64 never warms HAM).
    kpad_sb = []
    for h in range(HPC):
        t = p_qk.tile([128, S], BF16, tag=f"kpad{h}", name=f"kpad{h}")
        nc.vector.memset(t[64 * (1 - h % 2):64 * (1 - h % 2) + 64, :], 0.0)
        kpad_sb.append(t)
    vx_sb = [None] * NT  # [128, 4*65] bf16: per head 64 v-cols + ones col
    aT_sb = [p_aT.tile([128, S], BF16, tag=f"aT{i}", name=f"aT{i}")
             for i in range(2)]

    # ---- phase A: projections + rope ----
    with tc.tile_pool(name="ps_qk", bufs=4, space="PSUM") as ps_qk, \
         tc.tile_pool(name="ps_v", bufs=2, space="PSUM") as ps_v:
        with nc.named_scope("warmup"):
            # ~4us of dense K=128 matmuls (discarded) flips the PE clock
            # gate to 2.4 GHz while the x DMAs are still in flight
            wu = ps_v.tile([128, 512], F32, tag="v", name="warmup")
            for r in range(18):
                nc.tensor.matmul(wu[:, :], wqk_sb[0][:, 0:128],
                                 wqk_sb[0][:, :], start=(r == 0),
                                 stop=(r == 17), skip_group_check=True)
        def v_group(st_range):
            for st in st_range:
                v_ps = ps_v.tile([128, 256], F32, tag="v", name=f"v{st}")
                for kc in range(8):
                    nc.tensor.matmul(
                        v_ps[:, :],
                        x_sb[kc][:, ts(st, 128)],
                        wv_sb[kc][:, :],
                        start=(kc == 0), stop=(kc == 7))
                vx_t = p_vx.tile([128, HPC * 65], BF16, tag=f"vx{st}",
                                 name=f"vx{st}")
                vv = vx_t.rearrange("p (h c) -> p h c", c=65)
                nc.vector.memset(vv[:, :, 64:65], 1.0)
                nc.vector.tensor_add(
                    vv[:, :, 0:64],
                    v_ps.rearrange("p (h c) -> p h c", c=64)[:, :, :],
                    bvb_sb.rearrange("p (h c) -> p h c", c=64)[:, :, :])
                vx_sb[st] = vx_t

        with nc.named_scope("qk_proj"):
            for mc in (0, 2, -1, 1, 3, -2):  # -1/-2 = v-projection groups
                if mc == -1:
                    with nc.named_scope("v_proj_a"):
                        v_group(range(0, 8))
                    continue
                if mc == -2:
                    with nc.named_scope("v_proj_b"):
                        v_group(range(8, 16))
                    continue
                qk_pss = [ps_qk.tile([128, 512], F32, tag="qk",
                                     name=f"qk{mc}_{i}") for i in range(4)]
                if mc == 0:
                    # ns-outer: the first matmuls only need the first x
                    # chunks (input DMAs still in flight)
                    for ns in range(4):
                        for kc in range(8):
                            nc.tensor.matmul(
                                qk_pss[ns][:, :],
                                wqk_sb[kc][:, ts(mc, 128)],
                                x_sb[kc][:, ts(ns, 512)],
                                start=(kc == 0), stop=(kc == 7))
                else:
                    for kc in range(8):   # stationary wqk(kc, mc) reused
                        for ns in range(4):
                            nc.tensor.matmul(
                                qk_pss[ns][:, :],
                                wqk_sb[kc][:, ts(mc, 128)],
                                x_sb[kc][:, ts(ns, 512)],
                                start=(kc == 0), stop=(kc == 7))
                for ns in range(4):
                    qk_ps = qk_pss[ns]
                    raw = p_tmp.tile([128, 512], BF16, tag="raw")
                    nc.vector.tensor_scalar_add(raw[:, :], qk_ps[:, :],
                                                bqk_sb[mc][:, :])
                    # swap32: exchange adjacent 32-partition blocks
                    swp = p_tmp.tile([128, 512], BF16, tag="swp")
                    for blk, sb in ((0, 32), (32, 0), (64, 96), (96, 64)):
                        nc.sync.dma_start(swp[blk:blk + 32, :],
                                          raw[sb:sb + 32, :])
                    t1 = p_tmp.tile([128, 512], BF16, tag="t1")
                    nc.vector.tensor_mul(t1[:, :], swp[:, :], tabQ[:, ts(ns, 512)])
                    t2 = p_tmp.tile([128, 512], BF16, tag="t2")
                    nc.vector.tensor_mul(t2[:, :], raw[:, :], tabP[:, ts(ns, 512)])
                    if mc < 2:
                        nc.vector.tensor_add(
                            qk_sb[mc][:, ts(ns, 512)], t1[:, :], t2[:, :])
                    else:
                        hp = mc - 2
                        for hr in range(2):
                            nc.vector.tensor_add(
                                kpad_sb[2 * hp + hr][64 * hr:64 * hr + 64,
                                                     ts(ns, 512)],
                                t1[64 * hr:64 * hr + 64, :],
                                t2[64 * hr:64 * hr + 64, :])
    # ---- phase B: attention ----
    with tc.tile_pool(name="ps_s", bufs=2, space="PSUM") as ps_s, \
         tc.tile_pool(name="ps_o", bufs=2, space="PSUM") as ps_o:
        pend = [None]   # (p_t, j, q0, w, oT, qlo, h)

        def emit_pv(pv):
            p_t, j, q0, w, oT, qlo, h = pv
            c0 = (q0 - qlo) * 128
            pos = c0
            while pos < c0 + w:
                nxt = min((pos // 512 + 1) * 512, c0 + w)
                gbank = (qlo * 128 + pos) // 512
                nc.tensor.matmul(
                    oT[:, ds(pos, nxt - pos)],
                    vx_sb[j][:, ds(65 * h, 65)],
                    p_t[:, ds(pos - c0, nxt - pos)],
                    start=(j == 0), stop=(j == 4 * gbank + 3),
                    skip_group_check=True)
                pos = nxt

        def finish_half(h, hp, hr, half, oT):
            # Evacuate oT to SBUF right away so the PSUM banks free without
            # waiting on the (slow, single-lane) reciprocal; then normalize
            # from the copy in chunks, fully off the PE's path.
            num = p_r.tile([64, 1024], BF16, tag="num", name=f"num{h}_{half}")
            nc.vector.tensor_copy(num[:, :], oT[0:64, :])
            den = p_r.tile([1, 1024], F32, tag="den", name=f"den{h}_{half}")
            nc.vector.tensor_copy(den[:, :], oT[64:65, :])
            csz = 256 if (h == HPC - 1 and half == 1) else 512
            for c in range(1024 // csz):
                r_t = p_r.tile([1, 512], F32, tag="r", name=f"r{h}_{half}_{c}")
                nc.vector.reciprocal(r_t[:, 0:csz], den[:, ds(csz * c, csz)])
                rb_t = p_r.tile([64, 512], F32, tag="rb",
                                name=f"rb{h}_{half}_{c}")
                nc.gpsimd.partition_broadcast(rb_t[:, 0:csz], r_t[:, 0:csz])
                nc.vector.tensor_mul(
                    aT_sb[hp][64 * hr:64 * hr + 64,
                              ds(1024 * half + csz * c, csz)],
                    num[:, ds(csz * c, csz)], rb_t[:, 0:csz])

        finishes = []
        for h in range(HPC):
            hp, hr = h // 2, h % 2
            qT = qk_sb[hp]
            kT = kpad_sb[h]
            with nc.named_scope(f"attn_h{h}"):
                for half in range(2):
                    qlo, qhi = 8 * half, 8 * half + 8   # q-tile range
                    oT = ps_o.tile([65, 1024], F32, tag="oT",
                                   name=f"oT{h}_{half}")
                    for j in range(qhi):
                        q0 = max(j, qlo)
                        w = (qhi - q0) * 128
                        s_ps = ps_s.tile([128, 1024], F32, tag="s")
                        diag = (q0 == j)
                        for n0 in range(0, w, 512):
                            nn = min(512, w - n0)
                            has_mask = diag and n0 == 0
                            nc.tensor.matmul(
                                s_ps[:, ds(n0, nn)],
                                kT[:, ts(j, 128)],
                                qT[:, ds(q0 * 128 + n0, nn)],
                                start=True, stop=not has_mask,
                                skip_group_check=True)
                            if has_mask:
                                nc.tensor.matmul(
                                    s_ps[:, 0:128], neg_ident[:, :],
                                    upper01[:, :], start=False, stop=True,
                                    skip_group_check=True)
                        p_t = p_p.tile([128, 1024], BF16, tag="p")
                        nc.scalar.activation(
                            p_t[:, 0:w], s_ps[:, 0:w], AF.Exp, scale=SCALE)
                        if pend[0] is not None:
                            emit_pv(pend[0])
                            for fin in finishes:
                                fin()
                            finishes = []
                        pend[0] = (p_t, j, q0, w, oT, qlo, h)
                    finishes.append(
                        lambda h=h, hp=hp, hr=hr, half=half, oT=oT:
                            finish_half(h, hp, hr, half, oT))
        emit_pv(pend[0])
        for fin in finishes:
            fin()

    # ---- phase C: output projection ----
    with tc.tile_pool(name="ps_y", bufs=4, space="PSUM") as ps_y:
        with nc.named_scope("y_proj"):
            for qt in range(NT):
                for nh in range(2):
                    y_ps = ps_y.tile([128, 512], F32, tag="y")
                    for kc in range(2):
                        nc.tensor.matmul(
                            y_ps[:, :],
                            aT_sb[kc][:, ts(qt, 128)],
                            wout_sb[kc][:, ts(nh, 512)],
                            start=(kc == 0), stop=(kc == 1))
                    y_sb = p_y.tile([128, 512], F32, tag="ysb")
                    if nh == 0:
                        nc.vector.tensor_copy(y_sb[:, :], y_ps[:, :])
                    else:
                        nc.scalar.copy(y_sb[:, :], y_ps[:, :])
                    nc.sync.dma_start(y[ts(qt, 128), ts(nh, 512)], y_sb[:, :])


def build():
    nc = bacc.Bacc("TRN2", target_bir_lowering=False, debug=False,
                   num_devices=NCORES)
    xT = nc.dram_tensor("xT", [D, S], BF16, kind="ExternalInput").ap()
    wqk = nc.dram_tensor("wqk", [D, 512], BF16, kind="ExternalInput").ap()
    wv = nc.dram_tensor("wv", [D, 256], BF16, kind="ExternalInput").ap()
    wout = nc.dram_tensor("wout", [256, D], BF16, kind="ExternalInput").ap()
    bqk = nc.dram_tensor("bqk", [512], F32, kind="ExternalInput").ap()
    bvb = nc.dram_tensor("bvb", [128, 256], F32, kind="ExternalInput").ap()
    ropeP = nc.dram_tensor("ropeP", [128, S], BF16, kind="ExternalInput").ap()
    ropeQ = nc.dram_tensor("ropeQ", [128, S], BF16, kind="ExternalInput").ap()
    y = nc.dram_tensor("y", [S, D], F32, kind="ExternalOutput").ap()

    from contextlib import ExitStack
    with tile.TileContext(nc) as tc:
        with ExitStack() as ctx:
            _body(ctx, tc, (xT, wqk, wv, wout, bqk, bvb, ropeP, ropeQ), (y,))
    nc.compile()
    return nc


_EVEN_ODD = np.concatenate([np.arange(0, HD, 2), np.arange(1, HD, 2)])


def make_core_inputs(x, rope_cos, rope_sin, Wqkv, bqkv, Wout, bout, core):
    """Build the per-core device input map (numpy, host-side sharding)."""
    b, g = core // HPC, core % HPC
    heads = [HPC * g + i for i in range(HPC)]
    bf = ml_dtypes.bfloat16

    xT = np.ascontiguousarray(x[b].T).astype(bf)

    # wqk columns: [q01, q23, k01, k23]; within each head [evens, odds]
    qcols, kcols = [], []
    for h in heads:
        qcols.append(Wqkv[:, 0 * D + 64 * h + _EVEN_ODD])
        kcols.append(Wqkv[:, 1 * D + 64 * h + _EVEN_ODD])
    wqk_np = np.concatenate(
        [qcols[0], qcols[1], qcols[2], qcols[3],
         kcols[0], kcols[1], kcols[2], kcols[3]], axis=1)
    bq = [bqkv[0 * D + 64 * h + _EVEN_ODD] for h in heads]
    bk = [bqkv[1 * D + 64 * h + _EVEN_ODD] for h in heads]
    bqk_np = np.concatenate([bq[0], bq[1], bq[2], bq[3],
                             bk[0], bk[1], bk[2], bk[3]])

    wv_np = np.concatenate(
        [Wqkv[:, 2 * D + 64 * h:2 * D + 64 * h + 64] for h in heads], axis=1)
    bv = np.concatenate(
        [bqkv[2 * D + 64 * h:2 * D + 64 * h + 64] for h in heads])
    bvb_np = np.tile(bv[None, :], (128, 1)).astype(np.float32)

    wout_np = np.concatenate(
        [Wout[64 * h:64 * h + 64, :] for h in heads], axis=0)

    cosT = np.ascontiguousarray(rope_cos.T).astype(np.float32)  # [32, S]
    sinT = np.ascontiguousarray(rope_sin.T).astype(np.float32)
    ropeP_np = np.tile(np.concatenate([cosT, cosT], axis=0), (2, 1))
    ropeQ_np = np.tile(np.concatenate([-sinT, sinT], axis=0), (2, 1))

    return {
        "xT": xT,
        "wqk": np.ascontiguousarray(wqk_np).astype(bf),
        "wv": np.ascontiguousarray(wv_np).astype(bf),
        "wout": np.ascontiguousarray(wout_np).astype(bf),
        "bqk": bqk_np.astype(np.float32),
        "bvb": bvb_np,
        "ropeP": np.ascontiguousarray(ropeP_np).astype(bf),
        "ropeQ": np.ascontiguousarray(ropeQ_np).astype(bf),
    }


_NC_CACHE = None


def kernel(x, rope_cos, rope_sin, Wqkv, bqkv, Wout, bout):
    global _NC_CACHE, LAST_RESULTS
    x = np.asarray(x, dtype=np.float32)
    rope_cos = np.asarray(rope_cos, dtype=np.float32)
    rope_sin = np.asarray(rope_sin, dtype=np.float32)
    Wqkv = np.asarray(Wqkv, dtype=np.float32)
    bqkv = np.asarray(bqkv, dtype=np.float32)
    Wout = np.asarray(Wout, dtype=np.float32)
    bout = np.asarray(bout, dtype=np.float32)

    if _NC_CACHE is None:
        _NC_CACHE = build()
    nc = _NC_CACHE

    in_maps = [
        make_core_inputs(x, rope_cos, rope_sin, Wqkv, bqkv, Wout, bout, c)
        for c in range(NCORES)
    ]
    res = run_bass_kernel_spmd(nc, in_maps, core_ids=list(range(NCORES)),
                               trace=TRACE)
    LAST_RESULTS = res

    out = np.zeros((B, S, D), dtype=np.float32)
    for c in range(NCORES):
        out[c // HPC] += res.results[c]["y"]
    out += bout[None, None, :]
    return out


# revision 24
# speedup vs baseline: 1.0466x; 1.0248x over previous
# Causal self-attention kernel for 8 Trainium2 NeuronCores (Bass/Tile).
#
# Sharding: core c -> batch b = c//4, head group g = c%4 (heads 4g..4g+3).
# Each core computes the qkv projection for its batch restricted to its heads
# (column-sharded Wqkv), rope, causal flash attention for its 4 heads, and a
# row-sharded output projection producing a partial [S, D] f32 output.  The
# host sums the 4 partials per batch and adds bout.
#
# Device-side notes:
#  * All matmul inputs are bf16 (fp32 matmul is 4x slower on the PE); all
#    accumulation is f32 in PSUM.
#  * x is pre-transposed on host to xT [D, S] so the contraction dim lands on
#    SBUF partitions without any on-device transpose.
#  * q/k are produced directly transposed (qT/kT [dims, S]).  Within each head
#    the dims are permuted to [evens(32), odds(32)] so rope becomes
#    rot = x*P + swap32(x)*Q with host-built per-row tables P/Q; swap32 is two
#    32-partition-block exchanges done with SBUF->SBUF DMAs.
#  * Scores are computed transposed, sT[k, q], with the k-side stationary
#    operand zero-padded to K=128: K=64 matmuls never warm the PE clock gate
#    (HAM) and run at half clock forever.
#  * Causal masking of diagonal score tiles is one extra accumulating matmul
#    (-30000 * upper-triangle) -- no cross-engine dependency.
#  * Softmax without max-subtraction (scores ~ N(0,1); exp is safe in f32):
#    p = exp(s/8) straight out of PSUM on the scalar engine, bf16 out.
#  * v_ext [k, 65] carries a ones-column so the PV matmul accumulates the
#    softmax denominator as row 64 of oT [65, q].  oT is evacuated to SBUF
#    immediately (frees PSUM fast); reciprocal + gpsimd partition_broadcast +
#    multiply run off the PE's critical path.
#  * Attention runs in q-halves so oT is [65, 1024] = 2 PSUM banks and can
#    double-buffer; the PV matmul of iteration n is emitted after the scores
#    matmul of iteration n+1 so the PE never waits on the exp.
#  * Output projection: y[q, n] = sum_d aT[d, q] * Wout[d, n], q-tiled.

import numpy as np
import ml_dtypes

import concourse.bass as bass
import concourse.tile as tile
import concourse.mybir as mybir
from concourse import bacc
from concourse.bass import ts, ds
from concourse.bass_utils import run_bass_kernel_spmd

BF16 = mybir.dt.bfloat16
F32 = mybir.dt.float32
AF = mybir.ActivationFunctionType
ALU = mybir.AluOpType

B, S, D = 2, 2048, 1024
H, HD = 16, 64
NCORES = 8
HPC = 4            # heads per core
NT = S // 128      # 16 seq tiles
SCALE = HD ** -0.5
MASK_NEG = -30000.0

# Module-level knobs / results (used by test.py).
TRACE = False
LAST_RESULTS = None


def _body(ctx, tc, ins, outs):
    nc = tc.nc
    xT, wqk, wv, wout, bqk, bvb, ropeP, ropeQ = ins
    (y,) = outs

    # ---- SBUF pools ----
    p_x = ctx.enter_context(tc.tile_pool(name="x", bufs=1))
    p_w = ctx.enter_context(tc.tile_pool(name="w", bufs=1))
    p_cst = ctx.enter_context(tc.tile_pool(name="cst", bufs=1))
    p_qk = ctx.enter_context(tc.tile_pool(name="qk", bufs=1))
    p_vx = ctx.enter_context(tc.tile_pool(name="vx", bufs=1))
    p_aT = ctx.enter_context(tc.tile_pool(name="aT", bufs=1))
    p_tmp = ctx.enter_context(tc.tile_pool(name="tmp", bufs=3))
    p_p = ctx.enter_context(tc.tile_pool(name="p", bufs=6))
    p_r = ctx.enter_context(tc.tile_pool(name="r", bufs=2))
    p_y = ctx.enter_context(tc.tile_pool(name="y", bufs=3))

    # ---- load inputs (x and weight DMAs interleaved) ----
    x_sb, wqk_sb, wv_sb = [], [], []
    for kc in range(8):
        t = p_x.tile([128, S], BF16, tag=f"x{kc}", name=f"x{kc}")
        # issue x loads from the scalar engine's DMA queue: the sync
        # sequencer alone needs ~17us just to issue all input DMAs
        nc.scalar.dma_start(t[:, :], xT[ts(kc, 128), :])
        x_sb.append(t)
        t = p_w.tile([128, 512], BF16, tag=f"wqk{kc}", name=f"wqk{kc}")
        nc.sync.dma_start(t[:, :], wqk[ts(kc, 128), :])
        wqk_sb.append(t)
        t = p_w.tile([128, 256], BF16, tag=f"wv{kc}", name=f"wv{kc}")
        nc.sync.dma_start(t[:, :], wv[ts(kc, 128), :])
        wv_sb.append(t)
    wout_sb = []
    for kc in range(2):
        t = p_w.tile([128, D], BF16, tag=f"wout{kc}", name=f"wout{kc}")
        nc.sync.dma_start(t[:, :], wout[ts(kc, 128), :])
        wout_sb.append(t)
    tabP = p_cst.tile([128, S], BF16, tag="tabP")
    nc.sync.dma_start(tabP[:, :], ropeP[:, :])
    tabQ = p_cst.tile([128, S], BF16, tag="tabQ")
    nc.sync.dma_start(tabQ[:, :], ropeQ[:, :])
    bqk_sb = []
    for mc in range(4):
        t = p_cst.tile([128, 1], F32, tag=f"bqk{mc}", name=f"bqk{mc}")
        nc.sync.dma_start(t[:, :], bqk[ts(mc, 128)].rearrange("(p o) -> p o", o=1))
        bqk_sb.append(t)
    bvb_sb = p_cst.tile([128, 256], F32, tag="bvb")
    nc.sync.dma_start(bvb_sb[:, :], bvb[:, :])

    # constants for the matmul-based causal mask of diagonal score tiles:
    # s_diag += (neg_ident.T @ upper01) = -30000 where k > q.
    # (only is_ge is implemented for affine_select in this compiler)
    ones_t = p_cst.tile([128, 128], BF16, tag="ones")
    nc.vector.memset(ones_t[:, :], 1.0)
    upper01 = p_cst.tile([128, 128], BF16, tag="upper01")
    nc.gpsimd.affine_select(upper01[:, :], ones_t[:, :], pattern=[[-1, 128]],
                            compare_op=ALU.is_ge, fill=0.0, base=-1,
                            channel_multiplier=1)   # keep where k - q - 1 >= 0
    lower_t = p_cst.tile([128, 128], BF16, tag="lower")
    nc.gpsimd.affine_select(lower_t[:, :], ones_t[:, :], pattern=[[1, 128]],
                            compare_op=ALU.is_ge, fill=0.0, base=0,
                            channel_multiplier=-1)  # keep where q - k >= 0
    ident_t = p_cst.tile([128, 128], BF16, tag="ident")
    nc.gpsimd.affine_select(ident_t[:, :], lower_t[:, :], pattern=[[-1, 128]],
                            compare_op=ALU.is_ge, fill=0.0, base=0,
                            channel_multiplier=1)   # and k - q >= 0
    neg_ident = p_cst.tile([128, 128], BF16, tag="neg_ident")
    nc.vector.tensor_scalar_mul(neg_ident[:, :], ident_t[:, :], MASK_NEG)

    qk_sb = []   # [q01, q23, k01, k23], bf16 [128, S] each (post-rope)
    for mc in range(4):
        qk_sb.append(p_qk.tile([128, S], BF16, tag=f"qkT{mc}", name=f"qkT{mc}"))
    # per-head zero-padded kT [128, S]: only head h's 64 rows (at offset
    # 64*(h%2)) are nonzero, for full-K scores matmuls (K=64 never warms HAM).
    kpad_sb = []
    for h in range(HPC):
        t = p_qk.tile([128, S], BF16, tag=f"kpad{h}", name=f"kpad{h}")
        nc.vector.memset(t[64 * (1 - h % 2):64 * (1 - h % 2) + 64, :], 0.0)
        kpad_sb.append(t)
    vx_sb = [None] * NT  # [128, 4*65] bf16: per head 64 v-cols + ones col
    aT_sb = [p_aT.tile([128, S], BF16, tag=f"aT{i}", name=f"aT{i}")
             for i in range(2)]

    # ---- phase A: projections + rope ----
    with tc.tile_pool(name="ps_qk", bufs=4, space="PSUM") as ps_qk, \
         tc.tile_pool(name="ps_v", bufs=2, space="PSUM") as ps_v:
        with nc.named_scope("warmup"):
            # ~4us of dense K=128 matmuls (discarded) flips the PE clock
            # gate to 2.4 GHz while the x DMAs are still in flight
            wu = ps_v.tile([128, 512], F32, tag="v", name="warmup")
            for r in range(18):
                nc.tensor.matmul(wu[:, :], wqk_sb[0][:, 0:128],
                                 wqk_sb[0][:, :], start=(r == 0),
                                 stop=(r == 17), skip_group_check=True)
        def v_group(st_range):
            for st in st_range:
                v_ps = ps_v.tile([128, 256], F32, tag="v", name=f"v{st}")
                for kc in range(8):
                    nc.tensor.matmul(
                        v_ps[:, :],
                        x_sb[kc][:, ts(st, 128)],
                        wv_sb[kc][:, :],
                        start=(kc == 0), stop=(kc == 7))
                vx_t = p_vx.tile([128, HPC * 65], BF16, tag=f"vx{st}",
                                 name=f"vx{st}")
                vv = vx_t.rearrange("p (h c) -> p h c", c=65)
                nc.vector.memset(vv[:, :, 64:65], 1.0)
                nc.vector.tensor_add(
                    vv[:, :, 0:64],
                    v_ps.rearrange("p (h c) -> p h c", c=64)[:, :, :],
                    bvb_sb.rearrange("p (h c) -> p h c", c=64)[:, :, :])
                vx_sb[st] = vx_t

        with nc.named_scope("qk_proj"):
            for mc in (0, 2, -1, 1, 3, -2):  # -1/-2 = v-projection groups
                if mc == -1:
                    with nc.named_scope("v_proj_a"):
                        v_group(range(0, 8))
                    continue
                if mc == -2:
                    with nc.named_scope("v_proj_b"):
                        v_group(range(8, 16))
                    continue
                qk_pss = [ps_qk.tile([128, 512], F32, tag="qk",
                                     name=f"qk{mc}_{i}") for i in range(4)]
                if mc == 0:
                    # ns-outer: the first matmuls only need the first x
                    # chunks (input DMAs still in flight)
                    for ns in range(4):
                        for kc in range(8):
                            nc.tensor.matmul(
                                qk_pss[ns][:, :],
                                wqk_sb[kc][:, ts(mc, 128)],
                                x_sb[kc][:, ts(ns, 512)],
                                start=(kc == 0), stop=(kc == 7))
                else:
                    for kc in range(8):   # stationary wqk(kc, mc) reused
                        for ns in range(4):
                            nc.tensor.matmul(
                                qk_pss[ns][:, :],
                                wqk_sb[kc][:, ts(mc, 128)],
                                x_sb[kc][:, ts(ns, 512)],
                                start=(kc == 0), stop=(kc == 7))
                for ns in range(4):
                    qk_ps = qk_pss[ns]
                    raw = p_tmp.tile([128, 512], BF16, tag="raw")
                    nc.vector.tensor_scalar_add(raw[:, :], qk_ps[:, :],
                                                bqk_sb[mc][:, :])
                    # swap32: exchange adjacent 32-partition blocks
                    swp = p_tmp.tile([128, 512], BF16, tag="swp")
                    for blk, sb in ((0, 32), (32, 0), (64, 96), (96, 64)):
                        nc.sync.dma_start(swp[blk:blk + 32, :],
                                          raw[sb:sb + 32, :])
                    t1 = p_tmp.tile([128, 512], BF16, tag="t1")
                    nc.vector.tensor_mul(t1[:, :], swp[:, :], tabQ[:, ts(ns, 512)])
                    t2 = p_tmp.tile([128, 512], BF16, tag="t2")
                    nc.vector.tensor_mul(t2[:, :], raw[:, :], tabP[:, ts(ns, 512)])
                    if mc < 2:
                        nc.vector.tensor_add(
                            qk_sb[mc][:, ts(ns, 512)], t1[:, :], t2[:, :])
                    else:
                        hp = mc - 2
                        for hr in range(2):
                            nc.vector.tensor_add(
                                kpad_sb[2 * hp + hr][64 * hr:64 * hr + 64,
                                                     ts(ns, 512)],
                                t1[64 * hr:64 * hr + 64, :],
                                t2[64 * hr:64 * hr + 64, :])
    # ---- phase B: attention ----
    with tc.tile_pool(name="ps_s", bufs=2, space="PSUM") as ps_s, \
         tc.tile_pool(name="ps_o", bufs=2, space="PSUM") as ps_o:
        pend = [None]   # (p_t, j, q0, w, oT, qlo, h)

        def emit_pv(pv):
            p_t, j, q0, w, oT, qlo, h = pv
            c0 = (q0 - qlo) * 128
            pos = c0
            while pos < c0 + w:
                nxt = min((pos // 512 + 1) * 512, c0 + w)
                gbank = (qlo * 128 + pos) // 512
                nc.tensor.matmul(
                    oT[:, ds(pos, nxt - pos)],
                    vx_sb[j][:, ds(65 * h, 65)],
                    p_t[:, ds(pos - c0, nxt - pos)],
                    start=(j == 0), stop=(j == 4 * gbank + 3),
                    skip_group_check=True)
                pos = nxt

        def finish_half(h, hp, hr, half, oT):
            # Evacuate oT to SBUF right away so the PSUM banks free without
            # waiting on the (slow, single-lane) reciprocal; then normalize
            # from the copy in chunks, fully off the PE's path.
            num = p_r.tile([64, 1024], BF16, tag="num", name=f"num{h}_{half}")
            nc.vector.tensor_copy(num[:, :], oT[0:64, :])
            den = p_r.tile([1, 1024], F32, tag="den", name=f"den{h}_{half}")
            nc.vector.tensor_copy(den[:, :], oT[64:65, :])
            csz = 256 if (h == HPC - 1 and half == 1) else 512
            for c in range(1024 // csz):
                r_t = p_r.tile([1, 512], F32, tag="r", name=f"r{h}_{half}_{c}")
                nc.vector.reciprocal(r_t[:, 0:csz], den[:, ds(csz * c, csz)])
                rb_t = p_r.tile([64, 512], F32, tag="rb",
                                name=f"rb{h}_{half}_{c}")
                nc.gpsimd.partition_broadcast(rb_t[:, 0:csz], r_t[:, 0:csz])
                nc.vector.tensor_mul(
                    aT_sb[hp][64 * hr:64 * hr + 64,
                              ds(1024 * half + csz * c, csz)],
                    num[:, ds(csz * c, csz)], rb_t[:, 0:csz])

        finishes = []
        for h in range(HPC):
            hp, hr = h // 2, h % 2
            qT = qk_sb[hp]
            kT = kpad_sb[h]
            with nc.named_scope(f"attn_h{h}"):
                for half in range(2):
                    qlo, qhi = 8 * half, 8 * half + 8   # q-tile range
                    oT = ps_o.tile([65, 1024], F32, tag="oT",
                                   name=f"oT{h}_{half}")
                    for j in range(qhi):
                        q0 = max(j, qlo)
                        w = (qhi - q0) * 128
                        s_ps = ps_s.tile([128, 1024], F32, tag="s")
                        diag = (q0 == j)
                        for n0 in range(0, w, 512):
                            nn = min(512, w - n0)
                            has_mask = diag and n0 == 0
                            nc.tensor.matmul(
                                s_ps[:, ds(n0, nn)],
                                kT[:, ts(j, 128)],
                                qT[:, ds(q0 * 128 + n0, nn)],
                                start=True, stop=not has_mask,
                                skip_group_check=True)
                            if has_mask:
                                nc.tensor.matmul(
                                    s_ps[:, 0:128], neg_ident[:, :],
                                    upper01[:, :], start=False, stop=True,
                                    skip_group_check=True)
                        p_t = p_p.tile([128, 1024], BF16, tag="p")
                        nc.scalar.activation(
                            p_t[:, 0:w], s_ps[:, 0:w], AF.Exp, scale=SCALE)
                        if pend[0] is not None:
                            emit_pv(pend[0])
                            for fin in finishes:
                                fin()
                            finishes = []
                        pend[0] = (p_t, j, q0, w, oT, qlo, h)
                    finishes.append(
                        lambda h=h, hp=hp, hr=hr, half=half, oT=oT:
                            finish_half(h, hp, hr, half, oT))
        emit_pv(pend[0])
        for fin in finishes:
            fin()

    # ---- phase C: output projection ----
    with tc.tile_pool(name="ps_y", bufs=4, space="PSUM") as ps_y:
        with nc.named_scope("y_proj"):
            for qt in range(NT):
                # both 512-col halves land in one SBUF tile (DVE evacuates
                # one, ACT the other) so each q-tile needs a single store:
                # 16 store issues instead of 32 on the sync sequencer.
                y_sb = p_y.tile([128, 1024], F32, tag="ysb")
                for nh in range(2):
                    y_ps = ps_y.tile([128, 512], F32, tag="y")
                    for kc in range(2):
                        nc.tensor.matmul(
                            y_ps[:, :],
                            aT_sb[kc][:, ts(qt, 128)],
                            wout_sb[kc][:, ts(nh, 512)],
                            start=(kc == 0), stop=(kc == 1))
                    if nh == 0:
                        nc.vector.tensor_copy(y_sb[:, ts(nh, 512)], y_ps[:, :])
                    else:
                        nc.scalar.copy(y_sb[:, ts(nh, 512)], y_ps[:, :])
                nc.sync.dma_start(y[ts(qt, 128), :], y_sb[:, :])


def build():
    nc = bacc.Bacc("TRN2", target_bir_lowering=False, debug=False,
                   num_devices=NCORES)
    xT = nc.dram_tensor("xT", [D, S], BF16, kind="ExternalInput").ap()
    wqk = nc.dram_tensor("wqk", [D, 512], BF16, kind="ExternalInput").ap()
    wv = nc.dram_tensor("wv", [D, 256], BF16, kind="ExternalInput").ap()
    wout = nc.dram_tensor("wout", [256, D], BF16, kind="ExternalInput").ap()
    bqk = nc.dram_tensor("bqk", [512], F32, kind="ExternalInput").ap()
    bvb = nc.dram_tensor("bvb", [128, 256], F32, kind="ExternalInput").ap()
    ropeP = nc.dram_tensor("ropeP", [128, S], BF16, kind="ExternalInput").ap()
    ropeQ = nc.dram_tensor("ropeQ", [128, S], BF16, kind="ExternalInput").ap()
    y = nc.dram_tensor("y", [S, D], F32, kind="ExternalOutput").ap()

    from contextlib import ExitStack
    with tile.TileContext(nc) as tc:
        with ExitStack() as ctx:
            _body(ctx, tc, (xT, wqk, wv, wout, bqk, bvb, ropeP, ropeQ), (y,))
    nc.compile()
    return nc


_EVEN_ODD = np.concatenate([np.arange(0, HD, 2), np.arange(1, HD, 2)])


def make_core_inputs(x, rope_cos, rope_sin, Wqkv, bqkv, Wout, bout, core):
    """Build the per-core device input map (numpy, host-side sharding)."""
    b, g = core // HPC, core % HPC
    heads = [HPC * g + i for i in range(HPC)]
    bf = ml_dtypes.bfloat16

    xT = np.ascontiguousarray(x[b].T).astype(bf)

    # wqk columns: [q01, q23, k01, k23]; within each head [evens, odds]
    qcols, kcols = [], []
    for h in heads:
        qcols.append(Wqkv[:, 0 * D + 64 * h + _EVEN_ODD])
        kcols.append(Wqkv[:, 1 * D + 64 * h + _EVEN_ODD])
    wqk_np = np.concatenate(
        [qcols[0], qcols[1], qcols[2], qcols[3],
         kcols[0], kcols[1], kcols[2], kcols[3]], axis=1)
    bq = [bqkv[0 * D + 64 * h + _EVEN_ODD] for h in heads]
    bk = [bqkv[1 * D + 64 * h + _EVEN_ODD] for h in heads]
    bqk_np = np.concatenate([bq[0], bq[1], bq[2], bq[3],
                             bk[0], bk[1], bk[2], bk[3]])

    wv_np = np.concatenate(
        [Wqkv[:, 2 * D + 64 * h:2 * D + 64 * h + 64] for h in heads], axis=1)
    bv = np.concatenate(
        [bqkv[2 * D + 64 * h:2 * D + 64 * h + 64] for h in heads])
    bvb_np = np.tile(bv[None, :], (128, 1)).astype(np.float32)

    wout_np = np.concatenate(
        [Wout[64 * h:64 * h + 64, :] for h in heads], axis=0)

    cosT = np.ascontiguousarray(rope_cos.T).astype(np.float32)  # [32, S]
    sinT = np.ascontiguousarray(rope_sin.T).astype(np.float32)
    ropeP_np = np.tile(np.concatenate([cosT, cosT], axis=0), (2, 1))
    ropeQ_np = np.tile(np.concatenate([-sinT, sinT], axis=0), (2, 1))

    return {
        "xT": xT,
        "wqk": np.ascontiguousarray(wqk_np).astype(bf),
        "wv": np.ascontiguousarray(wv_np).astype(bf),
        "wout": np.ascontiguousarray(wout_np).astype(bf),
        "bqk": bqk_np.astype(np.float32),
        "bvb": bvb_np,
        "ropeP": np.ascontiguousarray(ropeP_np).astype(bf),
        "ropeQ": np.ascontiguousarray(ropeQ_np).astype(bf),
    }


_NC_CACHE = None


def kernel(x, rope_cos, rope_sin, Wqkv, bqkv, Wout, bout):
    global _NC_CACHE, LAST_RESULTS
    x = np.asarray(x, dtype=np.float32)
    rope_cos = np.asarray(rope_cos, dtype=np.float32)
    rope_sin = np.asarray(rope_sin, dtype=np.float32)
    Wqkv = np.asarray(Wqkv, dtype=np.float32)
    bqkv = np.asarray(bqkv, dtype=np.float32)
    Wout = np.asarray(Wout, dtype=np.float32)
    bout = np.asarray(bout, dtype=np.float32)

    if _NC_CACHE is None:
        _NC_CACHE = build()
    nc = _NC_CACHE

    in_maps = [
        make_core_inputs(x, rope_cos, rope_sin, Wqkv, bqkv, Wout, bout, c)
        for c in range(NCORES)
    ]
    res = run_bass_kernel_spmd(nc, in_maps, core_ids=list(range(NCORES)),
                               trace=TRACE)
    LAST_RESULTS = res

    out = np.zeros((B, S, D), dtype=np.float32)
    for c in range(NCORES):
        out[c // HPC] += res.results[c]["y"]
    out += bout[None, None, :]
    return out
